# revision 18
# baseline (speedup 1.0000x reference)
"""Bayesian triplet loss on 8 Trainium2 NeuronCores (Bass/Tile).

Data-parallel over the batch: each core owns BL=64 anchor rows.  The device
computes, per core, a packed [128, 512] PSUM block with FIVE N=512 matmul
passes (vs 9 unpacked):
   rows 0:64   g[i,j] = -2 e_i.e_j + n_j + BIGM*same - BIGM/2*diag
   rows 64:128 s[i,j] = -2 (u^2 e)_i.e_j + u^2_i.e_j^2
by packing the g- and s- lhsT operands side by side (M=128).  All lhsT
operands, the mask matrices, and the -2x/u^2 scalings are prepared on the
host (O(B*D) numpy) and shipped as one [128, 576] bf16 tensor; E^T ships as
two [128, 512] bf16 chunks; E^2 is squared on-chip.  A run of warm-up
matmuls on garbage SBUF runs during the DMA wait to lift the PE HAM clock
gate from 1.2 to 2.4 GHz before the real passes.

Mining runs as four fused DVE ops (no per-row tail on device):
   v1 ts(max-accum)  -> mxg   (+ free f32 copy of the s rows in its out)
   v2 ts(min-accum)  -> mng
   v3 stt((g==mxg) * s, sum-accum) -> selp
   v4 stt((g==mng) * s, sum-accum) -> seln
The row-constant n_i never touches the device (argmax/argmin are invariant
to it); the host adds n_i, c_i and computes the O(B) sqrt/softplus tail plus
the uncertainty-regularization term in numpy at f64.
"""

import numpy as np
import ml_dtypes

import concourse.bass as bass
import concourse.bacc as bacc
import concourse.mybir as mybir
import concourse.tile as tile
from concourse.bass_utils import run_bass_kernel_spmd
from contextlib import ExitStack

B, D, NCORES = 512, 256, 8
BL = B // NCORES              # anchors per core
F32 = mybir.dt.float32
BF16 = mybir.dt.bfloat16
OP = mybir.AluOpType

MARGIN, UW, MIN_U, MAX_U, EPS = 0.3, 0.05, 1e-6, 1.0, 1e-8
BIGM = 65536.0
NWARM = 5                     # PE warm-up matmuls issued during the DMA wait
OHR_ON_SWDGE = True           # ship the mask rhs on the gpsimd software-DGE queue


def _build_kernel(ctx: ExitStack, tc: "tile.TileContext", io: dict):
    nc = tc.nc
    sb = ctx.enter_context(tc.tile_pool(name="sb", bufs=1))
    ps = ctx.enter_context(tc.tile_pool(name="ps", bufs=1, space="PSUM"))

    # ---------- input DMAs (2 HWDGE queues + optionally SWDGE) ----------
    et0 = sb.tile([128, 512], BF16, tag="et0", name="et0")
    nc.sync.dma_start(et0[:], io["et0"][:])
    L = sb.tile([128, 576], BF16, tag="L", name="L")
    nc.scalar.dma_start(L[:], io["L"][:])
    et1 = sb.tile([128, 512], BF16, tag="et1", name="et1")
    nc.scalar.dma_start(et1[:], io["et1"][:])
    ohr = sb.tile([64, 512], BF16, tag="ohr", name="ohr")
    if OHR_ON_SWDGE:
        nc.gpsimd.dma_start(ohr[:], io["ohr"][:])
    else:
        nc.sync.dma_start(ohr[:], io["ohr"][:])

    # ---------- constants / warm-up ----------
    dum = sb.tile([128, 512], BF16, tag="dum", name="dum")
    nc.gpsimd.memset(dum[:], 1.0)
    stats = sb.tile([128, 4], F32, tag="stats", name="stats")
    nc.gpsimd.memset(stats[:], 0.0)
    psD = ps.tile([128, 512], F32, tag="psD", name="psD")
    for _ in range(NWARM):
        nc.tensor.matmul(psD[:], lhsT=dum[:, 0:128], rhs=dum[:], start=True,
                         stop=True)

    # ---------- on-chip E^2 ----------
    et2c0 = sb.tile([128, 512], BF16, tag="et2c0", name="et2c0")
    nc.vector.tensor_tensor(et2c0[:], et0[:], et0[:], OP.mult)
    et2c1 = sb.tile([128, 512], BF16, tag="et2c1", name="et2c1")
    nc.vector.tensor_tensor(et2c1[:], et1[:], et1[:], OP.mult)

    # ---------- packed matmuls: rows 0:64 = g, rows 64:128 = s ----------
    # The mask pass contracts K=64 (classes): rhs is onehotF with the
    # diagonal's columns scaled by 0.5, so BIGM*same - BIGM/2*diag comes out
    # of a single rank-64 product.
    psA = ps.tile([128, 512], F32, tag="psA", name="psA")
    nc.tensor.matmul(psA[:], lhsT=L[:, 0:128], rhs=et0[:], start=True, stop=False)
    nc.tensor.matmul(psA[:], lhsT=L[:, 128:256], rhs=et1[:], start=False, stop=False)
    nc.tensor.matmul(psA[:], lhsT=L[:, 256:384], rhs=et2c0[:], start=False,
                     stop=False)
    nc.tensor.matmul(psA[0:64, :], lhsT=L[0:64, 512:576], rhs=ohr[:], start=False,
                     stop=False)
    nc.tensor.matmul(psA[:], lhsT=L[:, 384:512], rhs=et2c1[:], start=False,
                     stop=True)

    # ---------- mining ----------
    # The BIR verifier requires all SBUF operands of an STT to share a base
    # partition, so the select ops run "at base 64": scalar copies of mxg/mng
    # are staged on partitions 64:128 (cross-partition single-src copies are
    # legal), in1 is junk1's s-half (already at 64:128), and out/accum land
    # on partitions 64:128.  Only the PSUM operand stays at base 0 (exempt).
    junk1 = sb.tile([128, 512], F32, tag="junk1", name="junk1")
    nc.vector.tensor_scalar(junk1[:], psA[:], 0.0, -3.0e38, OP.add, OP.max,
                            accum_out=stats[:, 0:1])
    junk2 = sb.tile([64, 512], F32, tag="junk2", name="junk2")
    nc.vector.tensor_scalar(junk2[:], psA[0:64, :], 0.0, 3.0e38, OP.add, OP.min,
                            accum_out=stats[0:64, 1:2])
    mxmn = sb.tile([128, 2], F32, tag="mxmn", name="mxmn")
    nc.vector.tensor_copy(mxmn[64:128, 0:1], stats[0:64, 0:1])
    nc.vector.tensor_copy(mxmn[64:128, 1:2], stats[0:64, 1:2])
    junk3 = sb.tile([128, 512], F32, tag="junk3", name="junk3")
    nc.vector.scalar_tensor_tensor(junk3[64:128, :], psA[0:64, :],
                                   mxmn[64:128, 0:1], junk1[64:128, :],
                                   OP.is_equal, OP.mult,
                                   accum_out=stats[64:128, 2:3])
    junk4 = sb.tile([128, 512], F32, tag="junk4", name="junk4")
    nc.vector.scalar_tensor_tensor(junk4[64:128, :], psA[0:64, :],
                                   mxmn[64:128, 1:2], junk1[64:128, :],
                                   OP.is_equal, OP.mult,
                                   accum_out=stats[64:128, 3:4])

    # ---------- output ----------
    nc.sync.dma_start(io["out"][:], stats[:])


_CACHE = {}


def _get_compiled():
    if "nc" in _CACHE:
        return _CACHE["nc"], _CACHE["io"]
    nc = bacc.Bacc("TRN2", target_bir_lowering=False, debug=False,
                   enable_asserts=False)
    io = {
        "et0": nc.dram_tensor("et0", [128, 512], BF16, kind="ExternalInput").ap(),
        "et1": nc.dram_tensor("et1", [128, 512], BF16, kind="ExternalInput").ap(),
        "L":   nc.dram_tensor("L",   [128, 576], BF16, kind="ExternalInput").ap(),
        "ohr": nc.dram_tensor("ohr", [64, 512], BF16, kind="ExternalInput").ap(),
        "out": nc.dram_tensor("out", [128, 4], F32, kind="ExternalOutput").ap(),
    }
    with tile.TileContext(nc) as tc, ExitStack() as ctx:
        _build_kernel(ctx, tc, io)
    nc.compile()
    _CACHE["nc"] = nc
    _CACHE["io"] = io
    return nc, io


def _clip_u(U):
    u = np.clip(U, MIN_U, MAX_U)
    return np.where(np.isnan(u) | np.isinf(u), MIN_U, u).astype(np.float32)


def _in_maps(E, U, labf):
    bf16 = ml_dtypes.bfloat16
    f = np.float32
    Eb = E.astype(bf16)
    ET = np.ascontiguousarray(Eb.T)                     # [256, 512]
    et0, et1 = np.ascontiguousarray(ET[0:128]), np.ascontiguousarray(ET[128:256])
    u = _clip_u(U)
    classes = np.arange(64, dtype=f)
    onehotF = (labf[None, :] == classes[:, None]).astype(f)     # [64, B]
    ones64 = np.ones((128, BL), f)
    maps = []
    for c in range(NCORES):
        c0 = c * BL
        Ec = E[c0:c0 + BL]
        ucx = u[c0:c0 + BL]
        neg2ecT = (-2.0 * Ec).T.reshape(2, 128, BL)             # [2,128,64]
        negatT = (-2.0 * (ucx * ucx) * Ec).T.reshape(2, 128, BL)
        u2T = (ucx * ucx).T.reshape(2, 128, BL)
        LA0 = np.concatenate([neg2ecT[0], negatT[0]], axis=1)
        LA1 = np.concatenate([neg2ecT[1], negatT[1]], axis=1)
        LB0 = np.concatenate([ones64, u2T[0]], axis=1)
        LB1 = np.concatenate([ones64, u2T[1]], axis=1)
        labc = labf[c0:c0 + BL]
        onehotC = (labc[None, :] == classes[:, None]).astype(f)  # [64,64]
        ohL = np.concatenate(
            [BIGM * onehotC, np.zeros((BL, BL), f)], axis=0)    # K rows 64:128 unused
        Lfull = np.concatenate([LA0, LA1, LB0, LB1, ohL], axis=1).astype(bf16)
        # Pure one-hot mask rhs: the diagonal needs no special term — a real
        # positive (d^2 > 0) always beats the diagonal (d^2 = 0) at argmax,
        # and the host flags no-positive rows via d_pos^2 < 100.
        ohr = onehotF.astype(bf16)
        maps.append({
            "et0": et0,
            "et1": et1,
            "L":   np.ascontiguousarray(Lfull),
            "ohr": np.ascontiguousarray(ohr),
        })
    return maps


def run_on_device(E, U, labf, trace=False, **kwargs):
    nc, _ = _get_compiled()
    maps = _in_maps(E, U, labf)
    res = run_bass_kernel_spmd(nc, maps, core_ids=list(range(NCORES)),
                               trace=trace, **kwargs)
    parts = np.stack([
        np.concatenate([np.asarray(r["out"])[0:BL, 0:2],
                        np.asarray(r["out"])[64:128, 2:4]], axis=1)
        for r in res.results])                                   # [8, 64, 4]
    return parts, res


def _finalize(parts, E, U):
    """Host tail: O(B) math on the per-row mined stats."""
    f = np.float64
    stats = parts.reshape(B, 4).astype(f)
    bf16 = ml_dtypes.bfloat16
    Eb = E.astype(bf16).astype(np.float32).astype(f)
    u = _clip_u(U).astype(f)
    n_i = (Eb * Eb).sum(axis=1)
    c_i = ((u * E.astype(f)) ** 2).sum(axis=1)
    mxg, mng, selp, seln = stats[:, 0], stats[:, 1], stats[:, 2], stats[:, 3]
    valid = (mxg + n_i - BIGM > 100.0) & (mng < 16384.0)
    d_pos = np.sqrt(np.maximum(mxg + n_i - BIGM, 0.0)) + EPS
    d_neg = np.sqrt(np.maximum(mng + n_i, 0.0)) + EPS
    u_pos2 = np.maximum(selp + c_i, 0.0) / (d_pos * d_pos) + EPS
    u_neg2 = np.maximum(seln + c_i, 0.0) / (d_neg * d_neg) + EPS
    sigma = np.sqrt(u_pos2 + u_neg2 + EPS)
    z = (d_pos - d_neg + MARGIN + UW * sigma) / sigma
    per = sigma * np.logaddexp(0.0, z)
    n_valid = max(float(valid.sum()), 1.0)
    total = float((per * valid).sum() / n_valid) + UW * float(u.mean())
    if np.isnan(total) or np.isinf(total):
        total = 0.0
    return np.float32(total)


def kernel(embeddings, uncertainties, labels):
    E = np.asarray(embeddings, dtype=np.float32)
    U = np.asarray(uncertainties, dtype=np.float32)
    labf = np.asarray(labels).astype(np.float32)
    parts, _ = run_on_device(E, U, labf)
    return _finalize(parts, E, U)


# revision 25
# speedup vs baseline: 1.3002x; 1.3002x over previous
"""Bayesian triplet loss on 8 Trainium2 NeuronCores (Bass/Tile).

Data-parallel over the batch: each core owns BL=64 anchor rows.  The device
computes, per core, a packed [128, 512] PSUM block with FIVE N=512 matmul
passes (vs 9 unpacked):
   rows 0:64   g[i,j] = -2 e_i.e_j + n_j + BIGM*same - BIGM/2*diag
   rows 64:128 s[i,j] = -2 (u^2 e)_i.e_j + u^2_i.e_j^2
by packing the g- and s- lhsT operands side by side (M=128).  All lhsT
operands, the mask matrices, and the -2x/u^2 scalings are prepared on the
host (O(B*D) numpy) and shipped as one [128, 576] bf16 tensor; E^T ships as
two [128, 512] bf16 chunks; E^2 is squared on-chip.  A run of warm-up
matmuls on garbage SBUF runs during the DMA wait to lift the PE HAM clock
gate from 1.2 to 2.4 GHz before the real passes.

Mining runs as four fused DVE ops (no per-row tail on device):
   v1 ts(max-accum)  -> mxg   (+ free f32 copy of the s rows in its out)
   v2 ts(min-accum)  -> mng
   v3 stt((g==mxg) * s, sum-accum) -> selp
   v4 stt((g==mng) * s, sum-accum) -> seln
The row-constant n_i never touches the device (argmax/argmin are invariant
to it); the host adds n_i, c_i and computes the O(B) sqrt/softplus tail plus
the uncertainty-regularization term in numpy at f64.
"""

import numpy as np
import ml_dtypes

import concourse.bass as bass
import concourse.bacc as bacc
import concourse.mybir as mybir
import concourse.tile as tile
from concourse.bass_utils import run_bass_kernel_spmd
from contextlib import ExitStack

B, D, NCORES = 512, 256, 8
BL = B // NCORES              # anchors per core
F32 = mybir.dt.float32
BF16 = mybir.dt.bfloat16
OP = mybir.AluOpType
AF = mybir.ActivationFunctionType

MARGIN, UW, MIN_U, MAX_U, EPS = 0.3, 0.05, 1e-6, 1.0, 1e-8
BIGM = 65536.0
NWARM = 10                    # PE warm-up matmuls issued during the DMA wait
OHR_ON_SWDGE = True           # ship the mask rhs on the gpsimd software-DGE queue


def _build_kernel(ctx: ExitStack, tc: "tile.TileContext", io: dict):
    nc = tc.nc
    sb = ctx.enter_context(tc.tile_pool(name="sb", bufs=1))
    ps = ctx.enter_context(tc.tile_pool(name="ps", bufs=1, space="PSUM"))

    # ---------- input DMAs (2 HWDGE queues + optionally SWDGE) ----------
    # A1 is gated on et0 + L, so each heads its own queue; et1 (the A2 gate)
    # streams right behind et0; the mask rhs rides the software queue.
    et0 = sb.tile([128, 512], BF16, tag="et0", name="et0")
    nc.sync.dma_start(et0[:], io["et0"][:])
    L = sb.tile([128, 576], BF16, tag="L", name="L")
    nc.scalar.dma_start(L[:], io["L"][:])
    et1 = sb.tile([128, 512], BF16, tag="et1", name="et1")
    nc.sync.dma_start(et1[:], io["et1"][:])
    ohr = sb.tile([64, 512], BF16, tag="ohr", name="ohr")
    if OHR_ON_SWDGE:
        nc.gpsimd.dma_start(ohr[:], io["ohr"][:])
    else:
        nc.scalar.dma_start(ohr[:], io["ohr"][:])

    # ---------- constants / warm-up ----------
    # N=256 dummy matmuls on garbage SBUF keep the PE busy through the DMA
    # wait so the HAM clock gate lifts (1.2 -> 2.4 GHz) before the real
    # passes. ~10 x ~300ns cold spans the ~3.4us busy window requirement.
    dum = sb.tile([128, 256], BF16, tag="dum", name="dum")
    nc.gpsimd.memset(dum[:], 1.0)
    stats = sb.tile([64, 4], F32, tag="stats", name="stats")
    psD = ps.tile([128, 256], F32, tag="psD", name="psD")
    for _ in range(NWARM):
        nc.tensor.matmul(psD[:], lhsT=dum[:, 0:128], rhs=dum[:], start=True,
                         stop=True)

    # ---------- on-chip E^2 ----------
    et2c0 = sb.tile([128, 512], BF16, tag="et2c0", name="et2c0")
    nc.vector.tensor_tensor(et2c0[:], et0[:], et0[:], OP.mult)
    et2c1 = sb.tile([128, 512], BF16, tag="et2c1", name="et2c1")
    nc.vector.tensor_tensor(et2c1[:], et1[:], et1[:], OP.mult)

    # ---------- packed matmuls: rows 0:64 = g, rows 64:128 = s ----------
    # The mask pass contracts K=64 (classes): rhs is onehotF with the
    # diagonal's columns scaled by 0.5, so BIGM*same - BIGM/2*diag comes out
    # of a single rank-64 product.
    psA = ps.tile([128, 512], F32, tag="psA", name="psA")
    nc.tensor.matmul(psA[:], lhsT=L[:, 0:128], rhs=et0[:], start=True, stop=False)
    nc.tensor.matmul(psA[:], lhsT=L[:, 128:256], rhs=et1[:], start=False, stop=False)
    nc.tensor.matmul(psA[:], lhsT=L[:, 256:384], rhs=et2c0[:], start=False,
                     stop=False)
    nc.tensor.matmul(psA[0:64, :], lhsT=L[0:64, 512:576], rhs=ohr[:], start=False,
                     stop=False)
    nc.tensor.matmul(psA[:], lhsT=L[:, 384:512], rhs=et2c1[:], start=False,
                     stop=True)

    # ---------- PSUM -> SBUF staging on the otherwise-idle ACT engine ----------
    # Mining then runs entirely on SBUF f32 at one partition base: the ts
    # reductions get the 2-port DVE mode and the STT selects need no
    # cross-base scalar staging.  Consistency holds because max/min and
    # is_equal all read the same ACT-written copies.
    gsb = sb.tile([64, 512], F32, tag="gsb", name="gsb")
    nc.scalar.activation(gsb[:], psA[0:64, :], AF.Identity)
    ssb = sb.tile([64, 512], F32, tag="ssb", name="ssb")
    nc.scalar.activation(ssb[:], psA[64:128, :], AF.Identity)

    # ---------- mining (all-SBUF, single partition base) ----------
    junk1 = sb.tile([64, 512], F32, tag="junk1", name="junk1")
    nc.vector.tensor_scalar(junk1[:], gsb[:], 0.0, -3.0e38, OP.add, OP.max,
                            accum_out=stats[:, 0:1])
    junk2 = sb.tile([64, 512], F32, tag="junk2", name="junk2")
    nc.vector.tensor_scalar(junk2[:], gsb[:], 0.0, 3.0e38, OP.add, OP.min,
                            accum_out=stats[:, 1:2])
    junk3 = sb.tile([64, 512], F32, tag="junk3", name="junk3")
    nc.vector.scalar_tensor_tensor(junk3[:], gsb[:], stats[:, 0:1], ssb[:],
                                   OP.is_equal, OP.mult,
                                   accum_out=stats[:, 2:3])
    junk4 = sb.tile([64, 512], F32, tag="junk4", name="junk4")
    nc.vector.scalar_tensor_tensor(junk4[:], gsb[:], stats[:, 1:2], ssb[:],
                                   OP.is_equal, OP.mult,
                                   accum_out=stats[:, 3:4])

    # ---------- output ----------
    nc.sync.dma_start(io["out"][:], stats[:])


_CACHE = {}


def _get_compiled():
    if "nc" in _CACHE:
        return _CACHE["nc"], _CACHE["io"]
    nc = bacc.Bacc("TRN2", target_bir_lowering=False, debug=False,
                   enable_asserts=False)
    io = {
        "et0": nc.dram_tensor("et0", [128, 512], BF16, kind="ExternalInput").ap(),
        "et1": nc.dram_tensor("et1", [128, 512], BF16, kind="ExternalInput").ap(),
        "L":   nc.dram_tensor("L",   [128, 576], BF16, kind="ExternalInput").ap(),
        "ohr": nc.dram_tensor("ohr", [64, 512], BF16, kind="ExternalInput").ap(),
        "out": nc.dram_tensor("out", [64, 4], F32, kind="ExternalOutput").ap(),
    }
    with tile.TileContext(nc) as tc, ExitStack() as ctx:
        _build_kernel(ctx, tc, io)
    nc.compile()
    _CACHE["nc"] = nc
    _CACHE["io"] = io
    return nc, io


def _clip_u(U):
    u = np.clip(U, MIN_U, MAX_U)
    return np.where(np.isnan(u) | np.isinf(u), MIN_U, u).astype(np.float32)


def _in_maps(E, U, labf):
    bf16 = ml_dtypes.bfloat16
    f = np.float32
    Eb = E.astype(bf16)
    ET = np.ascontiguousarray(Eb.T)                     # [256, 512]
    et0, et1 = np.ascontiguousarray(ET[0:128]), np.ascontiguousarray(ET[128:256])
    u = _clip_u(U)
    classes = np.arange(64, dtype=f)
    onehotF = (labf[None, :] == classes[:, None]).astype(f)     # [64, B]
    ones64 = np.ones((128, BL), f)
    maps = []
    for c in range(NCORES):
        c0 = c * BL
        Ec = E[c0:c0 + BL]
        ucx = u[c0:c0 + BL]
        neg2ecT = (-2.0 * Ec).T.reshape(2, 128, BL)             # [2,128,64]
        negatT = (-2.0 * (ucx * ucx) * Ec).T.reshape(2, 128, BL)
        u2T = (ucx * ucx).T.reshape(2, 128, BL)
        LA0 = np.concatenate([neg2ecT[0], negatT[0]], axis=1)
        LA1 = np.concatenate([neg2ecT[1], negatT[1]], axis=1)
        LB0 = np.concatenate([ones64, u2T[0]], axis=1)
        LB1 = np.concatenate([ones64, u2T[1]], axis=1)
        labc = labf[c0:c0 + BL]
        onehotC = (labc[None, :] == classes[:, None]).astype(f)  # [64,64]
        ohL = np.concatenate(
            [BIGM * onehotC, np.zeros((BL, BL), f)], axis=0)    # K rows 64:128 unused
        Lfull = np.concatenate([LA0, LA1, LB0, LB1, ohL], axis=1).astype(bf16)
        # Pure one-hot mask rhs: the diagonal needs no special term — a real
        # positive (d^2 > 0) always beats the diagonal (d^2 = 0) at argmax,
        # and the host flags no-positive rows via d_pos^2 < 100.
        ohr = onehotF.astype(bf16)
        maps.append({
            "et0": et0,
            "et1": et1,
            "L":   np.ascontiguousarray(Lfull),
            "ohr": np.ascontiguousarray(ohr),
        })
    return maps


def run_on_device(E, U, labf, trace=False, **kwargs):
    nc, _ = _get_compiled()
    maps = _in_maps(E, U, labf)
    res = run_bass_kernel_spmd(nc, maps, core_ids=list(range(NCORES)),
                               trace=trace, **kwargs)
    parts = np.stack([np.asarray(r["out"]) for r in res.results])  # [8, 64, 4]
    return parts, res


def _finalize(parts, E, U):
    """Host tail: O(B) math on the per-row mined stats."""
    f = np.float64
    stats = parts.reshape(B, 4).astype(f)
    bf16 = ml_dtypes.bfloat16
    Eb = E.astype(bf16).astype(np.float32).astype(f)
    u = _clip_u(U).astype(f)
    n_i = (Eb * Eb).sum(axis=1)
    c_i = ((u * E.astype(f)) ** 2).sum(axis=1)
    mxg, mng, selp, seln = stats[:, 0], stats[:, 1], stats[:, 2], stats[:, 3]
    valid = (mxg + n_i - BIGM > 100.0) & (mng < 16384.0)
    d_pos = np.sqrt(np.maximum(mxg + n_i - BIGM, 0.0)) + EPS
    d_neg = np.sqrt(np.maximum(mng + n_i, 0.0)) + EPS
    u_pos2 = np.maximum(selp + c_i, 0.0) / (d_pos * d_pos) + EPS
    u_neg2 = np.maximum(seln + c_i, 0.0) / (d_neg * d_neg) + EPS
    sigma = np.sqrt(u_pos2 + u_neg2 + EPS)
    z = (d_pos - d_neg + MARGIN + UW * sigma) / sigma
    per = sigma * np.logaddexp(0.0, z)
    n_valid = max(float(valid.sum()), 1.0)
    total = float((per * valid).sum() / n_valid) + UW * float(u.mean())
    if np.isnan(total) or np.isinf(total):
        total = 0.0
    return np.float32(total)


def kernel(embeddings, uncertainties, labels):
    E = np.asarray(embeddings, dtype=np.float32)
    U = np.asarray(uncertainties, dtype=np.float32)
    labf = np.asarray(labels).astype(np.float32)
    parts, _ = run_on_device(E, U, labf)
    return _finalize(parts, E, U)
